# revision 64
# baseline (speedup 1.0000x reference)
"""Trainium2 Bass kernel for nn_AttnBlock (GroupNorm + single-head spatial
self-attention + residual), SPMD over 8 NeuronCores.

Sharding: data-parallel over batch B=4, x2 split over query tokens
(each core handles 2048 of the 4096 spatial tokens of one batch).
The per-core input x-slab is permuted so that the core's own query tokens
come first -> the SPMD program is identical on every core (softmax/GN are
permutation-invariant over tokens).

Device algebra (weights fused on host in fp64):
  GroupNorm: xn = s*x + t   (s = rstd*gn_w, t = gn_b - mu*rstd*gn_w; stats
      computed on device from the fp8 copies of x: mean via PE ones-matmuls
      over x^T, E[x^2] via DVE/ACT square-reduce over x)
  r = W1 xn + rb,  W1 = Wk^T Wq, rb = Wk^T bq  (bk cancels in softmax)
  scoresT[j,i] = sum_c xn[c,j] r[c,i]
      = sum_c x[c,j]*(s_c r[c,i]) + D[i];  D[i] cancels in softmax exactly,
      so the keys stay RAW fp8 x and s folds into the r-evacuation scale.
      W1 is pre-scaled by s per input channel on device (W1TS), x64 to keep
      fp8 e4m3 in its normal range.
  e = exp(scoresT/sqrt(C) - 3)   (constant shift cancels in softmax; keeps
      e in fp8 range: max logit ~7.3 -> max e ~80 < 240)
  u_raw[c,i] = sum_j x[c,j] e[j,i];  denom[i] = sum_j e[j,i]  (values are
      RAW fp8 x^T, host-pretransposed: sum_j xn e = s*u_raw + t*denom)
  u8 = 16*(s*u_raw)/denom  (in fp8; the t-term and biases enter the proj
      PSUM through a rank-1 bias matmul)
  out = W2 u8/1024 + (b2 + W2 t) + x,  W2 = Wp Wv (x64 in fp8), b2 = Wp bv + bp

All heavy matmuls are fp8e4m3 with perf_mode=DoubleRow (K=256 per
instruction, [128,2,M]/[128,2,N] access patterns). No PE transposes and no
full-size normalize passes remain. End-to-end rel err ~6.7e-3 (gate 2e-2).
"""

import os
import sys

for _p in ("/opt/trn_rl_repo", "/root/.axon_site/_ro/trn_rl_repo"):
    if os.path.isdir(_p) and _p not in sys.path:
        sys.path.insert(0, _p)

import numpy as np

B, C, H, W = 4, 512, 64, 64
N = H * W            # 4096 tokens
NQ = N // 2          # 2048 query tokens per core
T = C // 128         # 4 channel tiles
JT = N // 128        # 32 key tiles
JP = JT // 2         # 16 DoubleRow key-tile pairs
IG = NQ // 512       # 4 query groups of 512
NUM_GROUPS = 32
EPS = 1e-5
SCALE = float(C) ** -0.5
SHIFT = 3.0          # exp(logit - SHIFT); cancels in softmax
W1S = 64.0           # fp8 range pre-scale of s*W1 (subnormal avoidance)
W2S = 64.0           # fp8 range pre-scale of W2
TS = 64.0            # fp8 range pre-scale of the GN shift vectors
U8S = 16.0           # fp8 range pre-scale of u8
PSC = W2S * U8S      # proj PSUM carries PSC*(W2 ubar + bias)

ULAG = int(os.environ.get("BASS_ULAG", "12"))  # u-matmul lag in pairs

_PROGRAM_CACHE = {}
LAST_RESULTS = None

_LDW_PATCHED = False


def _patch_ldw_opt():
    """walrus disables its LDWEIGHTS optimization by default; re-enable."""
    global _LDW_PATCHED
    if _LDW_PATCHED or os.environ.get("BASS_LDW_OPT", "0") == "0":
        return
    from concourse import bass_utils as _bu

    _orig = _bu.run_command

    def _patched(argv, **kw):
        argv = [
            ("--enable-ldw-opt=true" if a == "--enable-ldw-opt=false" else a)
            for a in argv
        ]
        return _orig(argv, **kw)

    _bu.run_command = _patched
    _LDW_PATCHED = True


def _build_program(repeat: int = 1):
    _patch_ldw_opt()
    import concourse.bass as bass
    import concourse.tile as tile
    from concourse import bacc, mybir

    f32 = mybir.dt.float32
    bf16 = mybir.dt.bfloat16
    f8 = mybir.dt.float8e4
    AF = mybir.ActivationFunctionType
    OP = mybir.AluOpType
    DR = mybir.MatmulPerfMode.DoubleRow

    nc = bacc.Bacc("TRN2")

    xb_d = nc.declare_dram_parameter("xb", [C, N], f32, isOutput=False)
    x8_d = nc.declare_dram_parameter("x8", [C, N], f8, isOutput=False)
    xt8_d = nc.declare_dram_parameter("xt8", [N, C], f8, isOutput=False)
    w1tb_d = nc.declare_dram_parameter("w1tb", [C, C], bf16, isOutput=False)
    w2t8_d = nc.declare_dram_parameter("w2t8", [C, C], f8, isOutput=False)
    cv_d = nc.declare_dram_parameter("cvec", [128, 4, T], f32, isOutput=False)
    b2r_d = nc.declare_dram_parameter("b2r", [128, T, 128], f32, isOutput=False)
    gi_d = nc.declare_dram_parameter("gi", [128, 8], f32, isOutput=False)
    git_d = nc.declare_dram_parameter("git", [8, 128], f32, isOutput=False)
    out_d = nc.declare_dram_parameter("out", [C, NQ], f32, isOutput=True)
    debug = os.environ.get("BASS_DEBUG_DUMP", "0") == "1"
    if debug:
        dbg_ss_d = nc.declare_dram_parameter("dbg_ss", [128, 2, T], f32, isOutput=True)
        dbg_perch_d = nc.declare_dram_parameter("dbg_perch", [128, 2, T], f32, isOutput=True)
        dbg_r8_d = nc.declare_dram_parameter("dbg_r8", [128, T, 512], f32, isOutput=True)
        dbg_e8_d = nc.declare_dram_parameter("dbg_e8", [128, 2, 512], f32, isOutput=True)
        dbg_u8_d = nc.declare_dram_parameter("dbg_u8", [128, T, 512], f32, isOutput=True)
        dbg_w1ts_d = nc.declare_dram_parameter("dbg_w1ts", [128, T, C], f8, isOutput=True)
        dbg_pr_d = nc.declare_dram_parameter("dbg_pr", [128, T, 512], f32, isOutput=True)

    def drmm(out, lhsT, rhs, start, stop):
        nc.tensor.matmul(out, lhsT, rhs, start=start, stop=stop, perf_mode=DR)

    with tile.TileContext(nc) as tc:
        with (
            tc.tile_pool(name="big", bufs=1) as pbig,
            tc.tile_pool(name="const", bufs=1) as pc,
            tc.tile_pool(name="stat", bufs=2) as pst,
            tc.tile_pool(name="rpool", bufs=2) as prr,
            tc.tile_pool(name="upool", bufs=2) as puu,
            tc.tile_pool(name="epool", bufs=ULAG + 1) as pee,
            tc.tile_pool(name="iopool", bufs=2) as pio,
            tc.tile_pool(name="psS", bufs=3, space="PSUM") as ppS,
            tc.tile_pool(name="psU", bufs=4, space="PSUM") as ppU,
            tc.tile_pool(name="psD", bufs=1, space="PSUM") as ppD,
        ):
            # ---- DMA: the cost model's DMA bandwidth is one serial pipe, so
            # transfer order is critical. fp8 tensors + consts first; fp32 x
            # (residual only) as per-query-group slabs, issued last.
            # All input DMA on the sync queue: one queue = fully controlled
            # transfer order in the serial DMA pipe (ACT's queue stays free
            # for its compute). Order: x8 (stats), xt8 (means + values),
            # consts, weights, then the first residual slab.
            X8 = pbig.tile([128, T, N], f8, tag="x8")
            x8_t = x8_d[:].rearrange("(t p) n -> p t n", p=128)
            for t in range(T):
                nc.sync.dma_start(out=X8[:, t, :], in_=x8_t[:, t, :])
            XT8 = pbig.tile([128, JT, C], f8, tag="xt8")
            xt8_t = xt8_d[:].rearrange("(t p) c -> p t c", p=128)
            for h in range(4):
                nc.sync.dma_start(
                    out=XT8[:, 8 * h : 8 * (h + 1), :],
                    in_=xt8_t[:, 8 * h : 8 * (h + 1), :],
                )
            CV = pc.tile([128, 4, T], f32)
            nc.sync.dma_start(out=CV, in_=cv_d[:])
            RB, B2U, GNW, GNB = CV[:, 0, :], CV[:, 1, :], CV[:, 2, :], CV[:, 3, :]
            GI = pc.tile([128, 8], f32)
            nc.sync.dma_start(out=GI, in_=gi_d[:])
            GIT = pc.tile([8, 128], f32)
            nc.sync.dma_start(out=GIT, in_=git_d[:])
            B2R = pc.tile([128, T, 128], f32)
            nc.sync.dma_start(out=B2R, in_=b2r_d[:])
            W1TB = pc.tile([128, T, C], bf16)
            nc.sync.dma_start(out=W1TB, in_=w1tb_d[:].rearrange("(t p) f -> p t f", p=128))
            W2T8 = pc.tile([128, T, C], f8)
            nc.sync.dma_start(out=W2T8, in_=w2t8_d[:].rearrange("(t p) f -> p t f", p=128))
            # residual x (fp32): group-g slab needed only at group g's proj.
            # Slab 0 last in the prologue; slabs 1..3 issued just-in-time at
            # group boundaries so they never cut ahead in the DMA pipe.
            X = pbig.tile([128, T, N], f32, tag="x")
            xb_t = xb_d[:].rearrange("(t p) n -> p t n", p=128)

            def emit_xslab(g):
                nc.sync.dma_start(
                    out=X[:, :, 512 * g : 512 * (g + 1)],
                    in_=xb_t[:, :, 512 * g : 512 * (g + 1)],
                )

            emit_xslab(0)

            # ---- device constants ----
            # [128, 2, 16] so the DoubleRow Ko-step is 16B (ISA: step%16==0)
            ONES8F = pc.tile([128, 2, 16], f8)
            nc.vector.memset(ONES8F, 1.0)
            ONES8 = ONES8F[:, :, 0:1]
            ONESRB = pc.tile([1, 128], bf16)
            nc.vector.memset(ONESRB, U8S)     # folds u8 = U8S*s*u/denom
            ONES128 = pc.tile([128, 512], bf16)
            nc.vector.memset(ONES128, 1.0)
            shiftT = pc.tile([128, 1], f32)
            nc.vector.memset(shiftT, -SHIFT)
            invPSC = pc.tile([128, 1], f32)
            nc.vector.memset(invPSC, 1.0 / PSC)
            epsT = pc.tile([8, 1], f32)
            nc.vector.memset(epsT, EPS)
            SS = pc.tile([128, 2, T], f32)    # (s, t) per channel
            SDIV = pc.tile([128, T], f32)     # s / W1S
            RBS = pc.tile([128, T], f32)      # s * (rb + W1 t)
            W1TS = pc.tile([128, T, C], f8)   # fp8(W1S * s * W1)
            TCS8 = pc.tile([128, T, 1], f8)   # fp8(TS * t / s)
            TC8 = pc.tile([128, T, 1], f8)    # fp8(TS * t)
            B2PT4 = pc.tile([128, T, 128], bf16)  # masked bias rows (PSC-scaled)
            nc.vector.memset(B2PT4, 0.0)

            # ---- GroupNorm stats ----
            # per-channel means for t2,t3: PE ones-matmuls over x^T (fp8)
            # (t0,t1 means come from bn_aggr below)
            Pmean = {m: ppU.tile([128, 1], f32, tag="Pu", name=f"Pmean{m}")
                     for m in range(2, T)}
            for jp in range(JP):
                for m in range(2, T):
                    drmm(Pmean[m], XT8[:, 2 * jp : 2 * jp + 2, 128 * m : 128 * (m + 1)],
                         ONES8, start=(jp == 0), stop=(jp == JP - 1))
            # per-channel E[x^2]: DVE bn_stats (t0,t1; also yields their means)
            # + ACT Square-accum (t2,t3). The custom-DVE tensor_tensor_reduce
            # crashes this HW path (NRT_EXEC_UNIT_UNRECOVERABLE) - avoid it.
            perch = pc.tile([128, 2, T], f32)  # (mean, E[x^2]) per channel
            for t in range(2):
                stats_t = pst.tile([128, 8, 6], f32, tag="stats", name=f"st{t}")
                for s8 in range(8):
                    nc.vector.bn_stats(out=stats_t[:, s8, :],
                                       in_=X8[:, t, 512 * s8 : 512 * (s8 + 1)])
                mv_t = pst.tile([128, 2], f32, tag="mv", name=f"mv{t}")
                nc.vector.bn_aggr(out=mv_t, in_=stats_t)
                nc.vector.tensor_copy(perch[:, 0, t : t + 1], mv_t[:, 0:1])
                nc.vector.tensor_mul(perch[:, 1, t : t + 1], mv_t[:, 0:1], mv_t[:, 0:1])
                nc.vector.tensor_add(perch[:, 1, t : t + 1], perch[:, 1, t : t + 1],
                                     mv_t[:, 1:2])
            junk3 = pst.tile([128, N], f8, tag="junk2", name="junk2")
            junk4 = pst.tile([128, N], f8, tag="junk2", name="junk3")
            for t in range(2, 4):
                nc.scalar.activation(
                    junk3 if t == 2 else junk4, X8[:, t, :], AF.Square,
                    accum_out=perch[:, 1, t : t + 1],
                )
            for m in range(2, T):
                nc.vector.tensor_scalar(
                    out=perch[:, 0, m : m + 1], in0=Pmean[m],
                    scalar1=1.0 / N, scalar2=None, op0=OP.mult,
                )
            nc.vector.tensor_scalar(
                out=perch[:, 1, 2:4], in0=perch[:, 1, 2:4],
                scalar1=1.0 / N, scalar2=None, op0=OP.mult,
            )
            # group reduce: GSp[(g),(mean,Ex2)] then broadcast back per channel
            GSp = ppD.tile([8, 2, T], f32, tag="psD", name="GSp")
            nc.tensor.matmul(GSp, GI, perch, start=True, stop=True)
            GB = pst.tile([8, 2, T], f32, tag="GB")
            tmpg = pst.tile([8, T], f32, tag="tmpg")
            nc.vector.tensor_copy(GB, GSp)
            nc.vector.tensor_mul(tmpg, GB[:, 0, :], GB[:, 0, :])
            nc.vector.tensor_sub(GB[:, 1, :], GB[:, 1, :], tmpg)  # group var
            nc.scalar.activation(GB[:, 1, :], GB[:, 1, :], AF.Sqrt, bias=epsT)
            # last Sqrt-set user: swap to the Exp table now (off the critical
            # path) instead of at the first real exp
            dmy = pst.tile([8, 1], f32, tag="dmy")
            nc.scalar.activation(dmy, epsT, AF.Exp)
            GBR = pst.tile([8, T], f32, tag="GBR")
            nc.vector.reciprocal(GBR, GB[:, 1, :])               # group rstd
            nc.vector.tensor_copy(GB[:, 1, :], GBR)
            PB = ppD.tile([128, 2, T], f32, tag="psD", name="PBx")
            nc.tensor.matmul(PB, GIT, GB, start=True, stop=True)
            tmpc = pst.tile([128, T], f32, tag="tmpc")
            nc.vector.tensor_mul(SS[:, 0, :], PB[:, 1, :], GNW)   # s
            nc.vector.tensor_mul(tmpc, PB[:, 0, :], SS[:, 0, :])
            nc.vector.tensor_sub(SS[:, 1, :], GNB, tmpc)          # t

            # ---- weight/bias folds ----
            SW1 = pst.tile([128, T], f32, tag="SW1")
            nc.vector.tensor_scalar(out=SW1, in0=SS[:, 0, :], scalar1=W1S,
                                    scalar2=None, op0=OP.mult)
            for t in range(T):
                eng = nc.vector if t % 2 == 0 else nc.gpsimd
                eng.tensor_scalar(
                    out=W1TS[:, t, :], in0=W1TB[:, t, :],
                    scalar1=SW1[:, t : t + 1], scalar2=None, op0=OP.mult,
                )
            nc.vector.tensor_scalar(out=SDIV, in0=SS[:, 0, :], scalar1=1.0 / W1S,
                                    scalar2=None, op0=OP.mult)
            recs = pst.tile([128, T], f32, tag="recs")
            nc.vector.reciprocal(recs, SS[:, 0, :])
            tmps = pst.tile([128, T], f32, tag="tmps")
            nc.vector.tensor_mul(tmps, SS[:, 1, :], recs)         # t/s
            with nc.allow_low_precision(reason="fp8 range-scaled GN shift vectors"):
                nc.vector.tensor_scalar(out=TCS8[:, :, 0], in0=tmps, scalar1=TS,
                                        scalar2=None, op0=OP.mult)
                nc.vector.tensor_scalar(out=TC8[:, :, 0], in0=SS[:, 1, :], scalar1=TS,
                                        scalar2=None, op0=OP.mult)
            # W1 t (for the r bias): PW1t = (W1S s W1)^T (TS t/s) / (W1S TS)
            PW1t = [ppU.tile([128, 1], f32, tag="Pu", name=f"PW1t{m}") for m in range(T)]
            for m in range(T):
                for t in range(T):
                    nc.tensor.matmul(PW1t[m], W1TS[:, t, 128 * m : 128 * (m + 1)],
                                     TCS8[:, t, :], start=(t == 0), stop=(t == T - 1))
                nc.vector.scalar_tensor_tensor(
                    out=RBS[:, m : m + 1], in0=PW1t[m], scalar=1.0 / (W1S * TS),
                    in1=RB[:, m : m + 1], op0=OP.mult, op1=OP.add,
                )
            nc.vector.tensor_mul(RBS, RBS, SS[:, 0, :])
            # W2 t rows (proj bias): row for block mo lands at partition 32*mo
            PW2T4 = ppD.tile([128, 128], f32, tag="psD", name="PW2T4")
            for mo in range(T):
                for t in range(T):
                    nc.tensor.matmul(PW2T4[32 * mo : 32 * mo + 1, :], TC8[:, t, :],
                                     W2T8[:, t, 128 * mo : 128 * (mo + 1)],
                                     start=(t == 0), stop=(t == T - 1),
                                     tile_position=(0, 32 * mo))
            with nc.allow_low_precision(reason="bias rows in bf16"):
                for mo in range(T):
                    nc.vector.scalar_tensor_tensor(
                        out=B2PT4[32 * mo : 32 * mo + 1, mo, :],
                        in0=PW2T4[32 * mo : 32 * mo + 1, :],
                        scalar=PSC / (W2S * TS),
                        in1=B2R[32 * mo : 32 * mo + 1, mo, :],
                        op0=OP.mult, op1=OP.add,
                    )

            # ---- attention ----
            def emit_r_one(g, r8, m, on_act, dbg_pr=None):
                """One r m-block: matmul pair into a pool slot + evacuation
                to fp8 r8 (r8 = s/W1S * Pr + s*(rb + W1 t))."""
                isl = slice(512 * g, 512 * (g + 1))
                Pr = ppS.tile([128, 512], f32, tag="psS", name=f"Pr{_rep}_{g}_{m}")
                for p in range(T // 2):
                    drmm(Pr, W1TS[:, 2 * p : 2 * p + 2, 128 * m : 128 * (m + 1)],
                         X8[:, 2 * p : 2 * p + 2, isl],
                         start=(p == 0), stop=(p == T // 2 - 1))
                if dbg_pr is not None:
                    nc.vector.tensor_copy(dbg_pr[:, m, :], Pr)
                if on_act:
                    nc.scalar.activation(
                        r8[:, m, :], Pr, AF.Identity,
                        bias=RBS[:, m : m + 1], scale=SDIV[:, m : m + 1],
                    )
                else:
                    nc.vector.tensor_scalar(
                        out=r8[:, m, :], in0=Pr,
                        scalar1=SDIV[:, m : m + 1], scalar2=RBS[:, m : m + 1],
                        op0=OP.mult, op1=OP.add,
                    )

            def emit_r(g, r8, dbg_pr=None):
                for m in range(T):
                    emit_r_one(g, r8, m, on_act=True, dbg_pr=dbg_pr)

            def emit_scores(g, j, r8):
                Ps = ppS.tile([128, 512], f32, tag="psS", name=f"Ps{_rep}_{g}_{j}")
                for p in range(T // 2):
                    drmm(Ps, X8[:, 2 * p : 2 * p + 2, 128 * j : 128 * (j + 1)],
                         r8[:, 2 * p : 2 * p + 2, :],
                         start=(p == 0), stop=(p == T // 2 - 1))
                return Ps

            def emit_u(g, jp, e8, Pu, Pden):
                for m in range(T):
                    drmm(Pu[m], XT8[:, 2 * jp : 2 * jp + 2, 128 * m : 128 * (m + 1)],
                         e8, start=(jp == 0), stop=(jp == JP - 1))
                drmm(Pden, ONES8, e8, start=(jp == 0), stop=(jp == JP - 1))

            _rep = -1

            def tail_head(g, Pden):
                """Boundary chain head: 1/denom + its broadcast matmul.
                Emitted right at the group end (before the next r) so the
                norm chain latency hides under the next group's first pairs."""
                rec = pio.tile([1, 512], bf16, tag="rec", name=f"rec{_rep}_{g}")
                with nc.allow_low_precision(reason="1/denom in bf16: ~5e-4 end-to-end"):
                    nc.vector.reciprocal(rec, Pden)
                Pb = ppD.tile([128, 512], f32, tag="psD", name=f"Pb{_rep}_{g}")
                nc.tensor.matmul(Pb, ONESRB, rec, start=True, stop=True)
                return Pb

            def tail_steps(g, Pu, Pb, u8name):
                rbc = pio.tile([128, 512], f32, tag="rbc", name=f"rbc{_rep}_{g}")
                nc.scalar.copy(rbc, Pb)  # ACT: idle at the boundary
                u8 = puu.tile([128, T, 512], f8, tag="u", name=u8name)
                for m in range(T):
                    nc.vector.scalar_tensor_tensor(
                        out=u8[:, m, :], in0=Pu[m], scalar=SS[:, 0, m : m + 1],
                        in1=rbc, op0=OP.mult, op1=OP.mult,
                    )
                return u8

            def emit_proj_mm(g, u8, mo, last=False):
                # psD bank: free between Pb(g) (rbc) and Pden(g+1) (first u
                # of the next group, which lags ULAG pairs). The last group
                # instead reuses the (now successor-free) Pu banks so its
                # four proj matmuls don't serialize through one bank.
                pool, tg = (ppU, "Pu") if last else (ppD, "psD")
                Pp = pool.tile([128, 512], f32, tag=tg, name=f"Pp{_rep}_{g}_{mo}")
                nc.tensor.matmul(Pp, B2PT4[:, mo, :], ONES128, start=True, stop=False)
                for p in range(T // 2):
                    drmm(Pp, W2T8[:, 2 * p : 2 * p + 2, 128 * mo : 128 * (mo + 1)],
                         u8[:, 2 * p : 2 * p + 2, :],
                         start=False, stop=(p == T // 2 - 1))
                return Pp

            def emit_proj_out(g, mo, Pp):
                isl = slice(512 * g, 512 * (g + 1))
                o = pio.tile([128, 512], f32, tag="o", name=f"o{_rep}_{g}_{mo}", bufs=4)
                nc.vector.scalar_tensor_tensor(
                    out=o, in0=Pp, scalar=invPSC, in1=X[:, mo, isl],
                    op0=OP.mult, op1=OP.add,
                )
                nc.sync.dma_start(out=out_d[128 * mo : 128 * (mo + 1), isl], in_=o)

            # group 0 r
            r8 = prr.tile([128, T, 512], f8, tag="r", name="r_init")
            if debug:
                nc.sync.dma_start(out=dbg_w1ts_d[:], in_=W1TS)
                dbg_pr = pio.tile([128, T, 512], f32, tag="dbgpr", name="dbgpr")
                emit_r(0, r8, dbg_pr=dbg_pr)
                nc.sync.dma_start(out=dbg_pr_d[:], in_=dbg_pr)
            else:
                emit_r(0, r8)

            if debug:
                nc.sync.dma_start(out=dbg_ss_d[:], in_=SS)
                nc.sync.dma_start(out=dbg_perch_d[:], in_=perch)
                dbg_r = pio.tile([128, T, 512], f32, tag="dbgr", name="dbgr")
                nc.vector.tensor_copy(dbg_r, r8)
                nc.sync.dma_start(out=dbg_r8_d[:], in_=dbg_r)

            prev = None  # (g, Pu, Pden) awaiting tail
            for _rep in range(repeat):
              for g in range(IG):
                  Pu = [ppU.tile([128, 512], f32, tag="Pu", name=f"Pu{_rep}_{g}_{m}")
                        for m in range(T)]
                  Pden = ppD.tile([1, 512], f32, tag="psD", name=f"Pden{_rep}_{g}")
                  e8s = {}
                  u8_prev = None
                  nxt = g + 1 if g + 1 < IG else (0 if _rep + 1 < repeat else None)
                  # the last group has no successor competing for PSUM banks:
                  # a short lag shortens the exposed tail chain
                  ulag = ULAG if nxt is not None else 2
                  nr8 = None
                  for jp in range(JP):
                      Ps0 = emit_scores(g, 2 * jp, r8)
                      Ps1 = emit_scores(g, 2 * jp + 1, r8)
                      if nxt is not None and jp in (4, 6, 8, 10):
                          # next group's r, spread over mid-group pairs: the
                          # matmuls slot into PE gaps and the evacuations run
                          # on the mid-group-idle DVE, keeping ACT exp-only
                          if jp == 4:
                              nr8 = prr.tile([128, T, 512], f8, tag="r",
                                             name=f"r{_rep}_{nxt}")
                              emit_xslab(nxt) if nxt > g else None
                          emit_r_one(nxt, nr8, (jp - 4) // 2, on_act=False)
                      if prev is not None and jp == 1:
                          # tail of the previous group: norm chain + proj,
                          # placed after this group's first scores
                          pg, pPu, pPb = prev
                          u8_prev = tail_steps(pg, pPu, pPb, f"u{_rep}_{pg}")
                          if debug and _rep == 0 and pg == 0:
                              dbg_u = pio.tile([128, T, 512], f32, tag="dbgu", name="dbgu")
                              nc.vector.tensor_copy(dbg_u, u8_prev)
                              nc.sync.dma_start(out=dbg_u8_d[:], in_=dbg_u)
                          for mo in range(T):
                              Pp = emit_proj_mm(pg, u8_prev, mo)
                              emit_proj_out(pg, mo, Pp)
                          prev = None
                      e8 = pee.tile([128, 2, 512], f8, tag="e", name=f"e{_rep}_{g}_{jp}")
                      nc.scalar.activation(e8[:, 0, :], Ps0, AF.Exp,
                                           bias=shiftT, scale=SCALE)
                      nc.scalar.activation(e8[:, 1, :], Ps1, AF.Exp,
                                           bias=shiftT, scale=SCALE)
                      e8s[jp] = e8
                      if debug and _rep == 0 and g == 0 and jp == 0:
                          dbg_e = pio.tile([128, 2, 512], f32, tag="dbge", name="dbge")
                          nc.vector.tensor_copy(dbg_e, e8)
                          nc.sync.dma_start(out=dbg_e8_d[:], in_=dbg_e)
                      if jp >= ulag:
                          emit_u(g, jp - ulag, e8s.pop(jp - ulag), Pu, Pden)
                  for jj in range(JP - ulag, JP):
                      emit_u(g, jj, e8s.pop(jj), Pu, Pden)
                  Pb = tail_head(g, Pden)
                  if nxt is not None:
                      prev = (g, Pu, Pb)
                      r8 = nr8
                  else:
                      u8 = tail_steps(g, Pu, Pb, f"u{_rep}_{g}")
                      for mo in range(T):
                          Pp = emit_proj_mm(g, u8, mo, last=True)
                          emit_proj_out(g, mo, Pp)

    nc.compile()
    return nc


def _host_inputs(x, gn_w, gn_b, wq, bq, wk, bk, wv, bv, wp, bp):
    """Host-side weight fusion (fp64) + per-core input maps."""
    import ml_dtypes

    f32 = np.float32
    f8 = ml_dtypes.float8_e4m3
    bf = ml_dtypes.bfloat16
    wq64, wk64, wv64, wp64 = (np.asarray(w, np.float64) for w in (wq, wk, wv, wp))
    w1t = (wq64.T @ wk64).astype(f32)                        # [c', c'']
    w2t = (wp64 @ wv64).T.astype(f32)                        # [c', c_out]
    rb = (wk64.T @ np.asarray(bq, np.float64)).astype(f32)   # [c'']
    b2 = (wp64 @ np.asarray(bv, np.float64) + np.asarray(bp, np.float64)).astype(f32)

    def tile_vec(v):
        return np.ascontiguousarray(np.asarray(v, f32).reshape(T, 128).T)

    def _b2_rows(b2v):
        rows = np.zeros((128, T, 128), f32)
        for mo in range(T):
            rows[32 * mo, mo, :] = PSC * np.asarray(b2v, f32)[128 * mo : 128 * (mo + 1)]
        return rows

    gs = C // NUM_GROUPS
    gi = np.zeros((128, 8), f32)
    git = np.zeros((8, 128), f32)
    for p in range(128):
        gi[p, p // gs] = 1.0 / gs
        git[p // gs, p] = 1.0

    cvec = np.ascontiguousarray(
        np.stack([tile_vec(rb), tile_vec(b2), tile_vec(gn_w), tile_vec(gn_b)], axis=1)
    )
    common = {
        "w1tb": w1t.astype(bf),
        "w2t8": np.ascontiguousarray(w2t * W2S).astype(f8),
        "cvec": cvec,
        "b2r": _b2_rows(b2),
        "gi": gi,
        "git": git,
    }

    x2 = np.asarray(x, f32).reshape(B, C, N)
    in_maps = []
    for core in range(8):
        b, s = divmod(core, 2)
        xb = x2[b]
        if s == 1:
            xb = np.concatenate([xb[:, NQ:], xb[:, :NQ]], axis=1)
        xb = np.ascontiguousarray(xb)
        m = dict(common)
        m["xb"] = xb
        m["x8"] = xb.astype(f8)
        m["xt8"] = np.ascontiguousarray(xb.T).astype(f8)
        in_maps.append(m)
    return in_maps


def kernel(**inputs):
    global LAST_RESULTS
    from concourse.bass_utils import run_bass_kernel_spmd

    key = "fp8dr2"
    if key not in _PROGRAM_CACHE:
        _PROGRAM_CACHE[key] = _build_program()
    nc = _PROGRAM_CACHE[key]

    in_maps = _host_inputs(**{k: np.asarray(v) for k, v in inputs.items()})
    trace = bool(int(os.environ.get("BASS_KERNEL_TRACE", "0")))
    res = run_bass_kernel_spmd(
        nc, in_maps, list(range(8)), trace=trace,
        trace_cores=list(range(8)) if trace else None,
    )
    LAST_RESULTS = res

    out = np.empty((B, C, N), np.float32)
    for core in range(8):
        b, s = divmod(core, 2)
        out[b, :, NQ * s : NQ * (s + 1)] = res.results[core]["out"]
    return out.reshape(B, C, H, W)


# revision 65
# speedup vs baseline: 2.6562x; 2.6562x over previous
"""Trainium2 Bass kernel for nn_AttnBlock (GroupNorm + single-head spatial
self-attention + residual), SPMD over 8 NeuronCores.

Sharding: data-parallel over batch B=4, x2 split over query tokens
(each core handles 2048 of the 4096 spatial tokens of one batch).
The per-core input x-slab is permuted so that the core's own query tokens
come first -> the SPMD program is identical on every core (softmax/GN are
permutation-invariant over tokens).

Device algebra (weights fused on host in fp64):
  GroupNorm: xn = s*x + t   (s = rstd*gn_w, t = gn_b - mu*rstd*gn_w; stats
      computed on device from the fp8 copies of x: mean via PE ones-matmuls
      over x^T, E[x^2] via DVE/ACT square-reduce over x)
  r = W1 xn + rb,  W1 = Wk^T Wq, rb = Wk^T bq  (bk cancels in softmax)
  scoresT[j,i] = sum_c xn[c,j] r[c,i]
      = sum_c x[c,j]*(s_c r[c,i]) + D[i];  D[i] cancels in softmax exactly,
      so the keys stay RAW fp8 x and s folds into the r-evacuation scale.
      W1 is pre-scaled by s per input channel on device (W1TS), x64 to keep
      fp8 e4m3 in its normal range.
  e = exp(scoresT/sqrt(C) - 3)   (constant shift cancels in softmax; keeps
      e in fp8 range: max logit ~7.3 -> max e ~80 < 240)
  u_raw[c,i] = sum_j x[c,j] e[j,i];  denom[i] = sum_j e[j,i]  (values are
      RAW fp8 x^T, host-pretransposed: sum_j xn e = s*u_raw + t*denom)
  u8 = 16*(s*u_raw)/denom  (in fp8; the t-term and biases enter the proj
      PSUM through a rank-1 bias matmul)
  out = W2 u8/1024 + (b2 + W2 t) + x,  W2 = Wp Wv (x64 in fp8), b2 = Wp bv + bp

All heavy matmuls are fp8e4m3 with perf_mode=DoubleRow (K=256 per
instruction, [128,2,M]/[128,2,N] access patterns). No PE transposes and no
full-size normalize passes remain. End-to-end rel err ~6.7e-3 (gate 2e-2).
"""

import os
import sys

for _p in ("/opt/trn_rl_repo", "/root/.axon_site/_ro/trn_rl_repo"):
    if os.path.isdir(_p) and _p not in sys.path:
        sys.path.insert(0, _p)

import numpy as np

B, C, H, W = 4, 512, 64, 64
N = H * W            # 4096 tokens
NQ = N // 2          # 2048 query tokens per core
T = C // 128         # 4 channel tiles
JT = N // 128        # 32 key tiles
JP = JT // 2         # 16 DoubleRow key-tile pairs
IG = NQ // 512       # 4 query groups of 512
NUM_GROUPS = 32
EPS = 1e-5
SCALE = float(C) ** -0.5
SHIFT = 3.0          # exp(logit - SHIFT); cancels in softmax
W1S = 64.0           # fp8 range pre-scale of s*W1 (subnormal avoidance)
W2S = 64.0           # fp8 range pre-scale of W2
TS = 64.0            # fp8 range pre-scale of the GN shift vectors
U8S = 16.0           # fp8 range pre-scale of u8
PSC = W2S * U8S      # proj PSUM carries PSC*(W2 ubar + bias)

ULAG = int(os.environ.get("BASS_ULAG", "12"))  # u-matmul lag in pairs

_PROGRAM_CACHE = {}
LAST_RESULTS = None

_LDW_PATCHED = False


def _patch_ldw_opt():
    """walrus disables its LDWEIGHTS optimization by default; re-enable."""
    global _LDW_PATCHED
    if _LDW_PATCHED or os.environ.get("BASS_LDW_OPT", "0") == "0":
        return
    from concourse import bass_utils as _bu

    _orig = _bu.run_command

    def _patched(argv, **kw):
        argv = [
            ("--enable-ldw-opt=true" if a == "--enable-ldw-opt=false" else a)
            for a in argv
        ]
        return _orig(argv, **kw)

    _bu.run_command = _patched
    _LDW_PATCHED = True


def _build_program(repeat: int = 1):
    _patch_ldw_opt()
    import concourse.bass as bass
    import concourse.tile as tile
    from concourse import bacc, mybir

    f32 = mybir.dt.float32
    bf16 = mybir.dt.bfloat16
    f8 = mybir.dt.float8e4
    AF = mybir.ActivationFunctionType
    OP = mybir.AluOpType
    DR = mybir.MatmulPerfMode.DoubleRow

    nc = bacc.Bacc("TRN2")

    xb_d = nc.declare_dram_parameter("xb", [C, N], f32, isOutput=False)
    x8_d = nc.declare_dram_parameter("x8", [C, N], f8, isOutput=False)
    xt8_d = nc.declare_dram_parameter("xt8", [N, C], f8, isOutput=False)
    w1tb_d = nc.declare_dram_parameter("w1tb", [C, C], bf16, isOutput=False)
    w2t8_d = nc.declare_dram_parameter("w2t8", [C, C], f8, isOutput=False)
    cv_d = nc.declare_dram_parameter("cvec", [128, 4, T], f32, isOutput=False)
    b2r_d = nc.declare_dram_parameter("b2r", [128, T, 128], f32, isOutput=False)
    gi_d = nc.declare_dram_parameter("gi", [128, 8], f32, isOutput=False)
    git_d = nc.declare_dram_parameter("git", [8, 128], f32, isOutput=False)
    out_d = nc.declare_dram_parameter("out", [C, NQ], f32, isOutput=True)
    debug = os.environ.get("BASS_DEBUG_DUMP", "0") == "1"
    if debug:
        dbg_ss_d = nc.declare_dram_parameter("dbg_ss", [128, 2, T], f32, isOutput=True)
        dbg_perch_d = nc.declare_dram_parameter("dbg_perch", [128, 2, T], f32, isOutput=True)
        dbg_r8_d = nc.declare_dram_parameter("dbg_r8", [128, T, 512], f32, isOutput=True)
        dbg_e8_d = nc.declare_dram_parameter("dbg_e8", [128, 2, 512], f32, isOutput=True)
        dbg_u8_d = nc.declare_dram_parameter("dbg_u8", [128, T, 512], f32, isOutput=True)
        dbg_w1ts_d = nc.declare_dram_parameter("dbg_w1ts", [128, T, C], f8, isOutput=True)
        dbg_pr_d = nc.declare_dram_parameter("dbg_pr", [128, T, 512], f32, isOutput=True)

    def drmm(out, lhsT, rhs, start, stop):
        nc.tensor.matmul(out, lhsT, rhs, start=start, stop=stop, perf_mode=DR)

    with tile.TileContext(nc) as tc:
        with (
            tc.tile_pool(name="big", bufs=1) as pbig,
            tc.tile_pool(name="const", bufs=1) as pc,
            tc.tile_pool(name="stat", bufs=2) as pst,
            tc.tile_pool(name="rpool", bufs=2) as prr,
            tc.tile_pool(name="upool", bufs=2) as puu,
            tc.tile_pool(name="epool", bufs=ULAG + 1) as pee,
            tc.tile_pool(name="iopool", bufs=2) as pio,
            tc.tile_pool(name="psS", bufs=3, space="PSUM") as ppS,
            tc.tile_pool(name="psU", bufs=4, space="PSUM") as ppU,
            tc.tile_pool(name="psD", bufs=1, space="PSUM") as ppD,
        ):
            # ---- DMA: the cost model's DMA bandwidth is one serial pipe, so
            # transfer order is critical. fp8 tensors + consts first; fp32 x
            # (residual only) as per-query-group slabs, issued last.
            # All input DMA on the sync queue: one queue = fully controlled
            # transfer order in the serial DMA pipe (ACT's queue stays free
            # for its compute). Order: x8 (stats), xt8 (means + values),
            # consts, weights, then the first residual slab.
            X8 = pbig.tile([128, T, N], f8, tag="x8")
            x8_t = x8_d[:].rearrange("(t p) n -> p t n", p=128)
            for t in range(T):
                nc.sync.dma_start(out=X8[:, t, :], in_=x8_t[:, t, :])
            XT8 = pbig.tile([128, JT, C], f8, tag="xt8")
            xt8_t = xt8_d[:].rearrange("(t p) c -> p t c", p=128)
            for h in range(4):
                nc.sync.dma_start(
                    out=XT8[:, 8 * h : 8 * (h + 1), :],
                    in_=xt8_t[:, 8 * h : 8 * (h + 1), :],
                )
            CV = pc.tile([128, 4, T], f32)
            nc.sync.dma_start(out=CV, in_=cv_d[:])
            RB, B2U, GNW, GNB = CV[:, 0, :], CV[:, 1, :], CV[:, 2, :], CV[:, 3, :]
            GI = pc.tile([128, 8], f32)
            nc.sync.dma_start(out=GI, in_=gi_d[:])
            GIT = pc.tile([8, 128], f32)
            nc.sync.dma_start(out=GIT, in_=git_d[:])
            B2R = pc.tile([128, T, 128], f32)
            nc.sync.dma_start(out=B2R, in_=b2r_d[:])
            W1TB = pc.tile([128, T, C], bf16)
            nc.sync.dma_start(out=W1TB, in_=w1tb_d[:].rearrange("(t p) f -> p t f", p=128))
            W2T8 = pc.tile([128, T, C], f8)
            nc.sync.dma_start(out=W2T8, in_=w2t8_d[:].rearrange("(t p) f -> p t f", p=128))
            # residual x (fp32): group-g slab needed only at group g's proj.
            # Slab 0 last in the prologue; slabs 1..3 issued just-in-time at
            # group boundaries so they never cut ahead in the DMA pipe.
            X = pbig.tile([128, T, N], f32, tag="x")
            xb_t = xb_d[:].rearrange("(t p) n -> p t n", p=128)

            def emit_xslab(g):
                nc.sync.dma_start(
                    out=X[:, :, 512 * g : 512 * (g + 1)],
                    in_=xb_t[:, :, 512 * g : 512 * (g + 1)],
                )

            emit_xslab(0)

            # ---- device constants ----
            # [128, 2, 16] so the DoubleRow Ko-step is 16B (ISA: step%16==0)
            ONES8F = pc.tile([128, 2, 16], f8)
            nc.vector.memset(ONES8F, 1.0)
            ONES8 = ONES8F[:, :, 0:1]
            ONESRB = pc.tile([1, 128], bf16)
            nc.vector.memset(ONESRB, U8S)     # folds u8 = U8S*s*u/denom
            ONES128 = pc.tile([128, 512], bf16)
            nc.vector.memset(ONES128, 1.0)
            shiftT = pc.tile([128, 1], f32)
            nc.vector.memset(shiftT, -SHIFT)
            invPSC = pc.tile([128, 1], f32)
            nc.vector.memset(invPSC, 1.0 / PSC)
            epsT = pc.tile([8, 1], f32)
            nc.vector.memset(epsT, EPS)
            SS = pc.tile([128, 2, T], f32)    # (s, t) per channel
            SDIV = pc.tile([128, T], f32)     # s / W1S
            RBS = pc.tile([128, T], f32)      # s * (rb + W1 t)
            W1TS = pc.tile([128, T, C], f8)   # fp8(W1S * s * W1)
            TCS8 = pc.tile([128, T, 1], f8)   # fp8(TS * t / s)
            TC8 = pc.tile([128, T, 1], f8)    # fp8(TS * t)
            B2PT4 = pc.tile([128, T, 128], bf16)  # masked bias rows (PSC-scaled)
            nc.vector.memset(B2PT4, 0.0)

            # ---- GroupNorm stats ----
            # per-channel means for t2,t3: PE ones-matmuls over x^T (fp8)
            # (t0,t1 means come from bn_aggr below)
            Pmean = {m: ppU.tile([128, 1], f32, tag="Pu", name=f"Pmean{m}")
                     for m in range(2, T)}
            for jp in range(JP):
                for m in range(2, T):
                    drmm(Pmean[m], XT8[:, 2 * jp : 2 * jp + 2, 128 * m : 128 * (m + 1)],
                         ONES8, start=(jp == 0), stop=(jp == JP - 1))
            # per-channel E[x^2]: DVE bn_stats (t0,t1; also yields their means)
            # + ACT Square-accum (t2,t3). The custom-DVE tensor_tensor_reduce
            # crashes this HW path (NRT_EXEC_UNIT_UNRECOVERABLE) - avoid it.
            perch = pc.tile([128, 2, T], f32)  # (mean, E[x^2]) per channel
            for t in range(2):
                stats_t = pst.tile([128, 8, 6], f32, tag="stats", name=f"st{t}")
                for s8 in range(8):
                    nc.vector.bn_stats(out=stats_t[:, s8, :],
                                       in_=X8[:, t, 512 * s8 : 512 * (s8 + 1)])
                mv_t = pst.tile([128, 2], f32, tag="mv", name=f"mv{t}")
                nc.vector.bn_aggr(out=mv_t, in_=stats_t)
                nc.vector.tensor_copy(perch[:, 0, t : t + 1], mv_t[:, 0:1])
                nc.vector.tensor_mul(perch[:, 1, t : t + 1], mv_t[:, 0:1], mv_t[:, 0:1])
                nc.vector.tensor_add(perch[:, 1, t : t + 1], perch[:, 1, t : t + 1],
                                     mv_t[:, 1:2])
            junk3 = pst.tile([128, N], f8, tag="junk2", name="junk2")
            junk4 = pst.tile([128, N], f8, tag="junk2", name="junk3")
            for t in range(2, 4):
                nc.scalar.activation(
                    junk3 if t == 2 else junk4, X8[:, t, :], AF.Square,
                    accum_out=perch[:, 1, t : t + 1],
                )
            for m in range(2, T):
                nc.vector.tensor_scalar(
                    out=perch[:, 0, m : m + 1], in0=Pmean[m],
                    scalar1=1.0 / N, scalar2=None, op0=OP.mult,
                )
            nc.vector.tensor_scalar(
                out=perch[:, 1, 2:4], in0=perch[:, 1, 2:4],
                scalar1=1.0 / N, scalar2=None, op0=OP.mult,
            )
            # group reduce: GSp[(g),(mean,Ex2)] then broadcast back per channel
            GSp = ppD.tile([8, 2, T], f32, tag="psD", name="GSp")
            nc.tensor.matmul(GSp, GI, perch, start=True, stop=True)
            GB = pst.tile([8, 2, T], f32, tag="GB")
            tmpg = pst.tile([8, T], f32, tag="tmpg")
            nc.vector.tensor_copy(GB, GSp)
            nc.vector.tensor_mul(tmpg, GB[:, 0, :], GB[:, 0, :])
            nc.vector.tensor_sub(GB[:, 1, :], GB[:, 1, :], tmpg)  # group var
            nc.scalar.activation(GB[:, 1, :], GB[:, 1, :], AF.Sqrt, bias=epsT)
            # last Sqrt-set user: swap to the Exp table now (off the critical
            # path) instead of at the first real exp
            dmy = pst.tile([8, 1], f32, tag="dmy")
            nc.scalar.activation(dmy, epsT, AF.Exp)
            GBR = pst.tile([8, T], f32, tag="GBR")
            nc.vector.reciprocal(GBR, GB[:, 1, :])               # group rstd
            nc.vector.tensor_copy(GB[:, 1, :], GBR)
            PB = ppD.tile([128, 2, T], f32, tag="psD", name="PBx")
            nc.tensor.matmul(PB, GIT, GB, start=True, stop=True)
            tmpc = pst.tile([128, T], f32, tag="tmpc")
            nc.vector.tensor_mul(SS[:, 0, :], PB[:, 1, :], GNW)   # s
            nc.vector.tensor_mul(tmpc, PB[:, 0, :], SS[:, 0, :])
            nc.vector.tensor_sub(SS[:, 1, :], GNB, tmpc)          # t

            # ---- weight/bias folds ----
            SW1 = pst.tile([128, T], f32, tag="SW1")
            nc.vector.tensor_scalar(out=SW1, in0=SS[:, 0, :], scalar1=W1S,
                                    scalar2=None, op0=OP.mult)
            for t in range(T):
                eng = nc.vector if t % 2 == 0 else nc.gpsimd
                eng.tensor_scalar(
                    out=W1TS[:, t, :], in0=W1TB[:, t, :],
                    scalar1=SW1[:, t : t + 1], scalar2=None, op0=OP.mult,
                )
            nc.vector.tensor_scalar(out=SDIV, in0=SS[:, 0, :], scalar1=1.0 / W1S,
                                    scalar2=None, op0=OP.mult)
            recs = pst.tile([128, T], f32, tag="recs")
            nc.vector.reciprocal(recs, SS[:, 0, :])
            tmps = pst.tile([128, T], f32, tag="tmps")
            nc.vector.tensor_mul(tmps, SS[:, 1, :], recs)         # t/s
            with nc.allow_low_precision(reason="fp8 range-scaled GN shift vectors"):
                nc.vector.tensor_scalar(out=TCS8[:, :, 0], in0=tmps, scalar1=TS,
                                        scalar2=None, op0=OP.mult)
                nc.vector.tensor_scalar(out=TC8[:, :, 0], in0=SS[:, 1, :], scalar1=TS,
                                        scalar2=None, op0=OP.mult)
            # W1 t (for the r bias): PW1t = (W1S s W1)^T (TS t/s) / (W1S TS)
            PW1t = [ppU.tile([128, 1], f32, tag="Pu", name=f"PW1t{m}") for m in range(T)]
            for m in range(T):
                for t in range(T):
                    nc.tensor.matmul(PW1t[m], W1TS[:, t, 128 * m : 128 * (m + 1)],
                                     TCS8[:, t, :], start=(t == 0), stop=(t == T - 1))
                nc.vector.scalar_tensor_tensor(
                    out=RBS[:, m : m + 1], in0=PW1t[m], scalar=1.0 / (W1S * TS),
                    in1=RB[:, m : m + 1], op0=OP.mult, op1=OP.add,
                )
            nc.vector.tensor_mul(RBS, RBS, SS[:, 0, :])
            # W2 t rows (proj bias): row for block mo lands at partition 32*mo
            PW2T4 = ppD.tile([128, 128], f32, tag="psD", name="PW2T4")
            for mo in range(T):
                for t in range(T):
                    nc.tensor.matmul(PW2T4[32 * mo : 32 * mo + 1, :], TC8[:, t, :],
                                     W2T8[:, t, 128 * mo : 128 * (mo + 1)],
                                     start=(t == 0), stop=(t == T - 1),
                                     tile_position=(0, 32 * mo))
            with nc.allow_low_precision(reason="bias rows in bf16"):
                for mo in range(T):
                    nc.vector.scalar_tensor_tensor(
                        out=B2PT4[32 * mo : 32 * mo + 1, mo, :],
                        in0=PW2T4[32 * mo : 32 * mo + 1, :],
                        scalar=PSC / (W2S * TS),
                        in1=B2R[32 * mo : 32 * mo + 1, mo, :],
                        op0=OP.mult, op1=OP.add,
                    )

            # ---- attention ----
            def emit_r_one(g, r8, m, on_act, dbg_pr=None):
                """One r m-block: matmul pair into a pool slot + evacuation
                to fp8 r8 (r8 = s/W1S * Pr + s*(rb + W1 t))."""
                isl = slice(512 * g, 512 * (g + 1))
                Pr = ppS.tile([128, 512], f32, tag="psS", name=f"Pr{_rep}_{g}_{m}")
                for p in range(T // 2):
                    drmm(Pr, W1TS[:, 2 * p : 2 * p + 2, 128 * m : 128 * (m + 1)],
                         X8[:, 2 * p : 2 * p + 2, isl],
                         start=(p == 0), stop=(p == T // 2 - 1))
                if dbg_pr is not None:
                    nc.vector.tensor_copy(dbg_pr[:, m, :], Pr)
                if on_act:
                    nc.scalar.activation(
                        r8[:, m, :], Pr, AF.Identity,
                        bias=RBS[:, m : m + 1], scale=SDIV[:, m : m + 1],
                    )
                else:
                    nc.vector.tensor_scalar(
                        out=r8[:, m, :], in0=Pr,
                        scalar1=SDIV[:, m : m + 1], scalar2=RBS[:, m : m + 1],
                        op0=OP.mult, op1=OP.add,
                    )

            def emit_r(g, r8, dbg_pr=None):
                for m in range(T):
                    emit_r_one(g, r8, m, on_act=True, dbg_pr=dbg_pr)

            def emit_scores(g, j, r8):
                Ps = ppS.tile([128, 512], f32, tag="psS", name=f"Ps{_rep}_{g}_{j}")
                for p in range(T // 2):
                    drmm(Ps, X8[:, 2 * p : 2 * p + 2, 128 * j : 128 * (j + 1)],
                         r8[:, 2 * p : 2 * p + 2, :],
                         start=(p == 0), stop=(p == T // 2 - 1))
                return Ps

            def emit_u(g, jp, e8, Pu, Pden):
                for m in range(T):
                    drmm(Pu[m], XT8[:, 2 * jp : 2 * jp + 2, 128 * m : 128 * (m + 1)],
                         e8, start=(jp == 0), stop=(jp == JP - 1))
                drmm(Pden, ONES8, e8, start=(jp == 0), stop=(jp == JP - 1))

            _rep = -1

            def tail_head(g, Pden):
                """Boundary chain head: 1/denom + its broadcast matmul.
                Emitted right at the group end (before the next r) so the
                norm chain latency hides under the next group's first pairs."""
                rec = pio.tile([1, 512], bf16, tag="rec", name=f"rec{_rep}_{g}")
                with nc.allow_low_precision(reason="1/denom in bf16: ~5e-4 end-to-end"):
                    nc.vector.reciprocal(rec, Pden)
                Pb = ppD.tile([128, 512], f32, tag="psD", name=f"Pb{_rep}_{g}")
                nc.tensor.matmul(Pb, ONESRB, rec, start=True, stop=True)
                return Pb

            def tail_steps(g, Pu, Pb, u8name):
                rbc = pio.tile([128, 512], f32, tag="rbc", name=f"rbc{_rep}_{g}")
                nc.scalar.copy(rbc, Pb)  # ACT: idle at the boundary
                u8 = puu.tile([128, T, 512], f8, tag="u", name=u8name)
                for m in range(T):
                    nc.vector.scalar_tensor_tensor(
                        out=u8[:, m, :], in0=Pu[m], scalar=SS[:, 0, m : m + 1],
                        in1=rbc, op0=OP.mult, op1=OP.mult,
                    )
                return u8

            def emit_proj_mm(g, u8, mo, last=False):
                # psD bank: free between Pb(g) (rbc) and Pden(g+1) (first u
                # of the next group, which lags ULAG pairs). The last group
                # instead reuses the (now successor-free) Pu banks so its
                # four proj matmuls don't serialize through one bank.
                pool, tg = (ppU, "Pu") if last else (ppD, "psD")
                Pp = pool.tile([128, 512], f32, tag=tg, name=f"Pp{_rep}_{g}_{mo}")
                nc.tensor.matmul(Pp, B2PT4[:, mo, :], ONES128, start=True, stop=False)
                for p in range(T // 2):
                    drmm(Pp, W2T8[:, 2 * p : 2 * p + 2, 128 * mo : 128 * (mo + 1)],
                         u8[:, 2 * p : 2 * p + 2, :],
                         start=False, stop=(p == T // 2 - 1))
                return Pp

            def emit_proj_out(g, mo, Pp):
                isl = slice(512 * g, 512 * (g + 1))
                o = pio.tile([128, 512], f32, tag="o", name=f"o{_rep}_{g}_{mo}", bufs=4)
                nc.vector.scalar_tensor_tensor(
                    out=o, in0=Pp, scalar=invPSC, in1=X[:, mo, isl],
                    op0=OP.mult, op1=OP.add,
                )
                nc.sync.dma_start(out=out_d[128 * mo : 128 * (mo + 1), isl], in_=o)

            # group 0 r
            r8 = prr.tile([128, T, 512], f8, tag="r", name="r_init")
            if debug:
                nc.sync.dma_start(out=dbg_w1ts_d[:], in_=W1TS)
                dbg_pr = pio.tile([128, T, 512], f32, tag="dbgpr", name="dbgpr")
                emit_r(0, r8, dbg_pr=dbg_pr)
                nc.sync.dma_start(out=dbg_pr_d[:], in_=dbg_pr)
            else:
                emit_r(0, r8)

            if debug:
                nc.sync.dma_start(out=dbg_ss_d[:], in_=SS)
                nc.sync.dma_start(out=dbg_perch_d[:], in_=perch)
                dbg_r = pio.tile([128, T, 512], f32, tag="dbgr", name="dbgr")
                nc.vector.tensor_copy(dbg_r, r8)
                nc.sync.dma_start(out=dbg_r8_d[:], in_=dbg_r)

            prev = None  # (g, Pu, Pden) awaiting tail
            for _rep in range(repeat):
              for g in range(IG):
                  Pu = [ppU.tile([128, 512], f32, tag="Pu", name=f"Pu{_rep}_{g}_{m}")
                        for m in range(T)]
                  Pden = ppD.tile([1, 512], f32, tag="psD", name=f"Pden{_rep}_{g}")
                  e8s = {}
                  u8_prev = None
                  nxt = g + 1 if g + 1 < IG else (0 if _rep + 1 < repeat else None)
                  # the last group has no successor competing for PSUM banks:
                  # a short lag shortens the exposed tail chain
                  ulag = ULAG if nxt is not None else 2
                  nr8 = None
                  for jp in range(JP):
                      Ps0 = emit_scores(g, 2 * jp, r8)
                      Ps1 = emit_scores(g, 2 * jp + 1, r8)
                      if nxt is not None and jp in (4, 6, 8, 10):
                          # next group's r, spread over mid-group pairs: the
                          # matmuls slot into PE gaps and the evacuations run
                          # on the mid-group-idle DVE, keeping ACT exp-only
                          if jp == 4:
                              nr8 = prr.tile([128, T, 512], f8, tag="r",
                                             name=f"r{_rep}_{nxt}")
                              emit_xslab(nxt) if nxt > g else None
                          emit_r_one(nxt, nr8, (jp - 4) // 2, on_act=False)
                      if prev is not None and jp == 1:
                          # tail of the previous group: norm chain + proj,
                          # placed after this group's first scores
                          pg, pPu, pPb = prev
                          u8_prev = tail_steps(pg, pPu, pPb, f"u{_rep}_{pg}")
                          if debug and _rep == 0 and pg == 0:
                              dbg_u = pio.tile([128, T, 512], f32, tag="dbgu", name="dbgu")
                              nc.vector.tensor_copy(dbg_u, u8_prev)
                              nc.sync.dma_start(out=dbg_u8_d[:], in_=dbg_u)
                          for mo in range(T):
                              Pp = emit_proj_mm(pg, u8_prev, mo)
                              emit_proj_out(pg, mo, Pp)
                          prev = None
                      e8 = pee.tile([128, 2, 512], f8, tag="e", name=f"e{_rep}_{g}_{jp}")
                      nc.scalar.activation(e8[:, 0, :], Ps0, AF.Exp,
                                           bias=shiftT, scale=SCALE)
                      nc.scalar.activation(e8[:, 1, :], Ps1, AF.Exp,
                                           bias=shiftT, scale=SCALE)
                      e8s[jp] = e8
                      if debug and _rep == 0 and g == 0 and jp == 0:
                          dbg_e = pio.tile([128, 2, 512], f32, tag="dbge", name="dbge")
                          nc.vector.tensor_copy(dbg_e, e8)
                          nc.sync.dma_start(out=dbg_e8_d[:], in_=dbg_e)
                      if jp >= ulag:
                          emit_u(g, jp - ulag, e8s.pop(jp - ulag), Pu, Pden)
                  for jj in range(JP - ulag, JP):
                      emit_u(g, jj, e8s.pop(jj), Pu, Pden)
                  Pb = tail_head(g, Pden)
                  if nxt is not None:
                      prev = (g, Pu, Pb)
                      r8 = nr8
                  else:
                      u8 = tail_steps(g, Pu, Pb, f"u{_rep}_{g}")
                      for mo in range(T):
                          Pp = emit_proj_mm(g, u8, mo, last=True)
                          emit_proj_out(g, mo, Pp)

    nc.compile()
    return nc


def _host_inputs(x, gn_w, gn_b, wq, bq, wk, bk, wv, bv, wp, bp):
    """Host-side weight fusion (fp64) + per-core input maps."""
    import ml_dtypes

    f32 = np.float32
    f8 = ml_dtypes.float8_e4m3
    bf = ml_dtypes.bfloat16
    wq64, wk64, wv64, wp64 = (np.asarray(w, np.float64) for w in (wq, wk, wv, wp))
    w1t = (wq64.T @ wk64).astype(f32)                        # [c', c'']
    w2t = (wp64 @ wv64).T.astype(f32)                        # [c', c_out]
    rb = (wk64.T @ np.asarray(bq, np.float64)).astype(f32)   # [c'']
    b2 = (wp64 @ np.asarray(bv, np.float64) + np.asarray(bp, np.float64)).astype(f32)

    def tile_vec(v):
        return np.ascontiguousarray(np.asarray(v, f32).reshape(T, 128).T)

    def _b2_rows(b2v):
        rows = np.zeros((128, T, 128), f32)
        for mo in range(T):
            rows[32 * mo, mo, :] = PSC * np.asarray(b2v, f32)[128 * mo : 128 * (mo + 1)]
        return rows

    gs = C // NUM_GROUPS
    gi = np.zeros((128, 8), f32)
    git = np.zeros((8, 128), f32)
    for p in range(128):
        gi[p, p // gs] = 1.0 / gs
        git[p // gs, p] = 1.0

    cvec = np.ascontiguousarray(
        np.stack([tile_vec(rb), tile_vec(b2), tile_vec(gn_w), tile_vec(gn_b)], axis=1)
    )
    common = {
        "w1tb": w1t.astype(bf),
        "w2t8": np.ascontiguousarray(w2t * W2S).astype(f8),
        "cvec": cvec,
        "b2r": _b2_rows(b2),
        "gi": gi,
        "git": git,
    }

    x2 = np.asarray(x, f32).reshape(B, C, N)
    in_maps = []
    for core in range(8):
        b, s = divmod(core, 2)
        xb = x2[b]
        if s == 1:
            xb = np.concatenate([xb[:, NQ:], xb[:, :NQ]], axis=1)
        xb = np.ascontiguousarray(xb)
        m = dict(common)
        m["xb"] = xb
        m["x8"] = xb.astype(f8)
        m["xt8"] = np.ascontiguousarray(xb.T).astype(f8)
        in_maps.append(m)
    return in_maps


def kernel(**inputs):
    global LAST_RESULTS
    from concourse.bass_utils import run_bass_kernel_spmd

    key = "fp8dr2"
    if key not in _PROGRAM_CACHE:
        _PROGRAM_CACHE[key] = _build_program()
    nc = _PROGRAM_CACHE[key]

    in_maps = _host_inputs(**{k: np.asarray(v) for k, v in inputs.items()})
    trace = bool(int(os.environ.get("BASS_KERNEL_TRACE", "0")))
    res = None
    for attempt in range(3):
        try:
            res = run_bass_kernel_spmd(
                nc, in_maps, list(range(8)), trace=trace,
                trace_cores=list(range(8)) if trace else None,
            )
            break
        except Exception:
            # transient NRT/axon device errors have been observed; retry
            if attempt == 2:
                raise
            import time as _time

            _time.sleep(5)
    LAST_RESULTS = res

    out = np.empty((B, C, N), np.float32)
    for core in range(8):
        b, s = divmod(core, 2)
        out[b, :, NQ * s : NQ * (s + 1)] = res.results[core]["out"]
    return out.reshape(B, C, H, W)
